# revision 1
# baseline (speedup 1.0000x reference)
"""Trainium2 Bass kernel for CrossAttention2d.

Reference computation (per batch b):
    q = conv_feat[b] (as [C, HW]) projected -> [HW, d], + q_b
    k, v = vit_feat[b] [N, D] projected -> [N, d], + biases
    attn = softmax(q @ k.T / sqrt(d))          [HW, N]
    o = attn @ v                               [HW, d]
    out = o @ out_w.T + out_b -> [C, HW]

Sharding: data-parallel over batch B=8 across the 8 NeuronCores; each core
computes one full batch element.

Device layout (all matmuls in bf16 with fp32 PSUM accumulation):
    Q^T [d, HW]   = q_wT.T @ conv        (conv natural [C, HW] layout)
    vitT [D, N]   = PE-transpose of vit
    K^T [d, N]    = k_wT.T @ vitT ; V^T likewise; V [N, d] = transpose(V^T)
    S^T [N, HW]   = (K^T chunk).T @ Q^T  (keys on partitions, queries free)
    E^T           = exp(S^T / 8)         (no max-subtraction needed: |s/8| < ~3)
    O'^T [d+1,HW] = V'.T @ E^T  with V' = [V, ones]  (row d = softmax denoms)
    out [C, HW]   = out_wT.T @ (O^T * recip(denoms)) + out_b
"""

import numpy as np

B = 8
C = 256
H = W = 64
HW = 4096
N = 1024
D = 768
d = 64

_CACHED_NC = None


def _build_nc():
    import concourse.mybir as mybir
    from concourse import bacc
    from concourse.masks import make_identity
    from concourse.tile import TileContext

    dt = mybir.dt
    f32 = dt.float32
    bf16 = dt.bfloat16
    Exp = mybir.ActivationFunctionType.Exp
    Ident = mybir.ActivationFunctionType.Identity
    mult = mybir.AluOpType.mult

    nc = bacc.Bacc(None)

    conv = nc.declare_dram_parameter("conv_feat", [C, HW], f32, isOutput=False)
    vit = nc.declare_dram_parameter("vit_feat", [N, D], f32, isOutput=False)
    q_w = nc.declare_dram_parameter("q_w", [d, C], f32, isOutput=False)
    q_b = nc.declare_dram_parameter("q_b", [d], f32, isOutput=False)
    k_w = nc.declare_dram_parameter("k_w", [d, D], f32, isOutput=False)
    k_b = nc.declare_dram_parameter("k_b", [d], f32, isOutput=False)
    v_w = nc.declare_dram_parameter("v_w", [d, D], f32, isOutput=False)
    v_b = nc.declare_dram_parameter("v_b", [d], f32, isOutput=False)
    out_w = nc.declare_dram_parameter("out_w", [C, d], f32, isOutput=False)
    out_b = nc.declare_dram_parameter("out_b", [C], f32, isOutput=False)
    out = nc.declare_dram_parameter("out", [C, HW], f32, isOutput=True)

    import os

    debug_taps = os.environ.get("BASS_DEBUG_TAPS") == "1"
    taps = {}
    if debug_taps:
        taps["dbg_qT"] = nc.declare_dram_parameter("dbg_qT", [d, HW], f32, isOutput=True)
        taps["dbg_kT"] = nc.declare_dram_parameter("dbg_kT", [d, N], f32, isOutput=True)
        taps["dbg_vT"] = nc.declare_dram_parameter("dbg_vT", [d, N], f32, isOutput=True)
        taps["dbg_vp"] = nc.declare_dram_parameter(
            "dbg_vp", [128, 8, 65], f32, isOutput=True
        )
        taps["dbg_e0"] = nc.declare_dram_parameter(
            "dbg_e0", [128, HW], f32, isOutput=True
        )
        taps["dbg_vitT"] = nc.declare_dram_parameter(
            "dbg_vitT", [128, 6, N], f32, isOutput=True
        )
        taps["dbg_on"] = nc.declare_dram_parameter(
            "dbg_on", [8, 65, 512], f32, isOutput=True
        )
        taps["dbg_r"] = nc.declare_dram_parameter(
            "dbg_r", [8, 1, 512], f32, isOutput=True
        )
        taps["dbg_rb"] = nc.declare_dram_parameter(
            "dbg_rb", [8, 64, 512], f32, isOutput=True
        )
        taps["dbg_ot"] = nc.declare_dram_parameter(
            "dbg_ot", [8, 64, 512], f32, isOutput=True
        )

    with TileContext(nc) as tc:
        with (
            tc.tile_pool(name="const", bufs=1) as const,
            tc.tile_pool(name="data", bufs=1) as data,
            tc.tile_pool(name="work", bufs=4) as work,
        ):
            # ---- constants / weights -------------------------------------
            identity = const.tile([128, 128], bf16)
            make_identity(nc, identity)

            # raw weights, cast to bf16 during DMA (gpsimd SWDGE casts)
            wq_raw = const.tile([d, C], bf16)
            nc.gpsimd.dma_start(wq_raw, q_w[:, :])
            wk_raw = const.tile([d, D], bf16)
            nc.gpsimd.dma_start(wk_raw, k_w[:, :])
            wv_raw = const.tile([d, D], bf16)
            nc.gpsimd.dma_start(wv_raw, v_w[:, :])
            wo_raw = const.tile([128, 2, d], bf16)
            nc.gpsimd.dma_start(wo_raw, out_w.rearrange("(t p) e -> p t e", p=128))

            qb_sb = const.tile([d, 1], f32)
            nc.sync.dma_start(qb_sb, q_b.rearrange("(a b) -> a b", b=1))
            kb_sb = const.tile([d, 1], f32)
            nc.sync.dma_start(kb_sb, k_b.rearrange("(a b) -> a b", b=1))
            vb_sb = const.tile([d, 1], f32)
            nc.sync.dma_start(vb_sb, v_b.rearrange("(a b) -> a b", b=1))
            ob_sb = const.tile([128, 2], f32)
            nc.sync.dma_start(ob_sb, out_b.rearrange("(t p) -> p t", p=128))

            # ---- batch data loads (cast fp32 -> bf16 during DMA) ---------
            # vit first: its transposes are the first PE work that gates S
            vit_sb = data.tile([128, 8, D], bf16)
            vit_r = vit.rearrange("(c p) e -> p c e", p=128)
            nc.gpsimd.dma_start(vit_sb[:, 0:4, :], vit_r[:, 0:4, :])
            nc.gpsimd.dma_start(vit_sb[:, 4:8, :], vit_r[:, 4:8, :])
            conv_sb = data.tile([128, 2, HW], bf16)
            conv_r = conv.rearrange("(t p) f -> p t f", p=128)
            nc.gpsimd.dma_start(conv_sb[:, 0:1, :], conv_r[:, 0:1, :])
            nc.gpsimd.dma_start(conv_sb[:, 1:2, :], conv_r[:, 1:2, :])

            # persistent per-batch tensors
            qT_sb = data.tile([128, HW], bf16)  # rows 64-127 duplicate 0-63
            vitT_sb = data.tile([128, 6, N], bf16)
            kT_sb = data.tile([128, N], bf16)  # rows 64-127 duplicate 0-63
            vT_sb = data.tile([d, N], bf16)
            v_sb = data.tile([128, 8, 65], bf16)  # V' = [V, ones]
            e_sb = data.tile([128, 8, HW], bf16)
            out_sb = data.tile([128, 2, HW], f32)

            nc.vector.memset(v_sb[:, :, 64:65], 1.0)

            # ---- phase A: transposes + projections -----------------------
            with (
                tc.tile_pool(name="ptr", bufs=4, space="PSUM") as ptr,
                tc.tile_pool(name="pproj", bufs=4, space="PSUM") as pproj,
            ):
                # transpose the small weights on-device
                wqT = const.tile([128, 2, d], bf16)
                for t in range(2):
                    ps = ptr.tile([128, d], bf16, tag="tr")
                    nc.tensor.transpose(
                        ps, wq_raw[:, t * 128 : (t + 1) * 128], identity[0:d, 0:d]
                    )
                    nc.vector.tensor_copy(wqT[:, t, :], ps)
                wkT = const.tile([128, 6, d], bf16)
                wvT = const.tile([128, 6, d], bf16)
                for c in range(6):
                    ps = ptr.tile([128, d], bf16, tag="tr")
                    nc.tensor.transpose(
                        ps, wk_raw[:, c * 128 : (c + 1) * 128], identity[0:d, 0:d]
                    )
                    nc.vector.tensor_copy(wkT[:, c, :], ps)
                    ps = ptr.tile([128, d], bf16, tag="tr")
                    nc.tensor.transpose(
                        ps, wv_raw[:, c * 128 : (c + 1) * 128], identity[0:d, 0:d]
                    )
                    nc.vector.tensor_copy(wvT[:, c, :], ps)
                woT = const.tile([d, 2, 128], bf16)
                for t in range(2):
                    ps = ptr.tile([d, 128], bf16, tag="tr")
                    nc.tensor.transpose(ps, wo_raw[:, t, :], identity)
                    nc.vector.tensor_copy(woT[:, t, :], ps)

                # vit transpose on PE: [n, D] -> vitT [D, n], 3 column-blocks
                # per PSUM tile to batch the PSUM->SBUF copies
                for nch in range(8):
                    for dg in range(2):
                        pst = ptr.tile([128, 3, 128], bf16, tag="tr")
                        for k in range(3):
                            dch = dg * 3 + k
                            nc.tensor.transpose(
                                pst[:, k, :],
                                vit_sb[:, nch, dch * 128 : (dch + 1) * 128],
                                identity,
                            )
                        for k in range(3):
                            dch = dg * 3 + k
                            nc.vector.tensor_copy(
                                vitT_sb[:, dch, nch * 128 : (nch + 1) * 128],
                                pst[:, k, :],
                            )

                # Q^T = q_wT.T @ conv  (+q_b)
                for j in range(8):
                    qp = pproj.tile([d, 512], f32, tag="proj")
                    for t in range(2):
                        nc.tensor.matmul(
                            qp,
                            wqT[:, t, :],
                            conv_sb[:, t, j * 512 : (j + 1) * 512],
                            start=(t == 0),
                            stop=(t == 1),
                        )
                    nc.scalar.activation(
                        qT_sb[0:d, j * 512 : (j + 1) * 512], qp, func=Ident, bias=qb_sb
                    )
                    nc.vector.tensor_scalar_add(
                        qT_sb[d:128, j * 512 : (j + 1) * 512], qp, qb_sb
                    )

                # K^T, V^T = w.T @ vitT  (+biases)
                for h in range(2):
                    kp = pproj.tile([d, 512], f32, tag="proj")
                    for c in range(6):
                        nc.tensor.matmul(
                            kp,
                            wkT[:, c, :],
                            vitT_sb[:, c, h * 512 : (h + 1) * 512],
                            start=(c == 0),
                            stop=(c == 5),
                        )
                    nc.scalar.activation(
                        kT_sb[0:d, h * 512 : (h + 1) * 512], kp, func=Ident, bias=kb_sb
                    )
                    nc.vector.tensor_scalar_add(
                        kT_sb[d:128, h * 512 : (h + 1) * 512], kp, kb_sb
                    )
                    vp = pproj.tile([d, 512], f32, tag="proj")
                    for c in range(6):
                        nc.tensor.matmul(
                            vp,
                            wvT[:, c, :],
                            vitT_sb[:, c, h * 512 : (h + 1) * 512],
                            start=(c == 0),
                            stop=(c == 5),
                        )
                    nc.scalar.activation(
                        vT_sb[:, h * 512 : (h + 1) * 512], vp, func=Ident, bias=vb_sb
                    )

                # V [n, d] = transpose(V^T) on PE
                for c in range(8):
                    pst = ptr.tile([128, d], bf16, tag="tr")
                    nc.tensor.transpose(
                        pst, vT_sb[:, c * 128 : (c + 1) * 128], identity[0:d, 0:d]
                    )
                    nc.vector.tensor_copy(v_sb[:, c, 0:64], pst)

            # ---- phase B: S^T = K^T.T @ Q^T, E^T = exp(S^T/8) -------------
            with tc.tile_pool(name="ps", bufs=2, space="PSUM") as ps_pool:
                for c in range(8):
                    base = d * (c % 2)
                    for h in range(2):
                        sp = ps_pool.tile([128, 2048], f32, tag="s")
                        for jj in range(4):
                            j = h * 4 + jj
                            nc.tensor.matmul(
                                sp[:, jj * 512 : (jj + 1) * 512],
                                kT_sb[base : base + d, c * 128 : (c + 1) * 128],
                                qT_sb[base : base + d, j * 512 : (j + 1) * 512],
                                start=True,
                                stop=True,
                            )
                        nc.scalar.activation(
                            e_sb[:, c, h * 2048 : (h + 1) * 2048],
                            sp,
                            func=Exp,
                            scale=0.125,
                        )

            # ---- phase C: O'^T = V'.T @ E^T, normalize, out-projection ----
            with (
                tc.tile_pool(name="po", bufs=4, space="PSUM") as po,
                tc.tile_pool(name="pf", bufs=4, space="PSUM") as pf,
            ):
                for g in range(2):
                    ops = []
                    for jj in range(4):
                        ops.append(po.tile([65, 512], f32, tag="o", name=f"op{jj}"))
                    for c in range(8):
                        for jj in range(4):
                            j = g * 4 + jj
                            nc.tensor.matmul(
                                ops[jj],
                                v_sb[:, c, :],
                                e_sb[:, c, j * 512 : (j + 1) * 512],
                                start=(c == 0),
                                stop=(c == 7),
                            )
                    for jj in range(4):
                        j = g * 4 + jj
                        op = ops[jj]
                        r0 = work.tile([1, 512], f32, tag="r0")
                        nc.vector.tensor_copy(r0, op[64:65, :])
                        r = work.tile([1, 512], f32, tag="r")
                        nc.vector.reciprocal_approx_fast(r, r0)
                        rb = work.tile([64, 512], f32, tag="rb")
                        nc.gpsimd.partition_broadcast(rb, r)
                        ot = work.tile([64, 512], bf16, tag="ot")
                        nc.vector.tensor_tensor(ot, op[0:64, :], rb, mult)
                        if debug_taps:
                            onc = work.tile([65, 512], f32, tag="onc")
                            nc.vector.tensor_copy(onc, op)
                            nc.gpsimd.dma_start(taps["dbg_on"][j, :, :], onc)
                            nc.gpsimd.dma_start(taps["dbg_r"][j, :, :], r)
                            nc.gpsimd.dma_start(taps["dbg_rb"][j, :, :], rb)
                            nc.gpsimd.dma_start(taps["dbg_ot"][j, :, :], ot)
                        for t in range(2):
                            fp = pf.tile([128, 512], f32, tag="f")
                            nc.tensor.matmul(
                                fp, woT[:, t, :], ot, start=True, stop=True
                            )
                            nc.vector.tensor_scalar_add(
                                out_sb[:, t, j * 512 : (j + 1) * 512],
                                fp,
                                ob_sb[:, t : t + 1],
                            )
                    for t in range(2):
                        nc.sync.dma_start(
                            out[t * 128 : (t + 1) * 128, g * 2048 : (g + 1) * 2048],
                            out_sb[:, t, g * 2048 : (g + 1) * 2048],
                        )

            if debug_taps:
                nc.gpsimd.dma_start(taps["dbg_qT"][:, :], qT_sb)
                nc.gpsimd.dma_start(taps["dbg_kT"][:, :], kT_sb)
                nc.gpsimd.dma_start(taps["dbg_vT"][:, :], vT_sb)
                nc.gpsimd.dma_start(taps["dbg_vp"][:, :, :], v_sb)
                nc.gpsimd.dma_start(taps["dbg_e0"][:, :], e_sb[:, 0, :])
                nc.gpsimd.dma_start(taps["dbg_vitT"][:, :, :], vitT_sb)

    nc.finalize()
    return nc


def _get_nc():
    global _CACHED_NC
    if _CACHED_NC is None:
        _CACHED_NC = _build_nc()
    return _CACHED_NC


def kernel(**inputs) -> np.ndarray:
    from concourse.bass_utils import run_bass_kernel_spmd

    conv_feat = np.asarray(inputs["conv_feat"], dtype=np.float32)
    vit_feat = np.asarray(inputs["vit_feat"], dtype=np.float32)
    weights = {
        name: np.ascontiguousarray(np.asarray(inputs[name], dtype=np.float32))
        for name in ("q_w", "q_b", "k_w", "k_b", "v_w", "v_b", "out_w", "out_b")
    }

    nc = _get_nc()
    in_maps = []
    for b in range(B):
        m = dict(weights)
        m["conv_feat"] = np.ascontiguousarray(conv_feat[b].reshape(C, HW))
        m["vit_feat"] = np.ascontiguousarray(vit_feat[b])
        in_maps.append(m)

    res = run_bass_kernel_spmd(nc, in_maps, list(range(B)))
    return np.stack(
        [res.results[b]["out"].reshape(C, H, W) for b in range(B)]
    ).astype(np.float32)

